# revision 23
# baseline (speedup 1.0000x reference)
"""BiMambaLayer Trainium2 kernel (v2).

Sharding: 8 cores = batch(2) x direction(2) x head-half(2). Each core runs the
full L=2048 sequence of one (batch, direction) through 16 of the 32 heads of
that direction's Mamba2 block, plus the fused output projection restricted to
its 1024 d_inner channels. The gated-RMSNorm row scale commutes with the output
projections, so each core returns an unnormalized partial projection U and a
partial sum-of-squares s; the host combines:
    out[b] = x[b] + scale * sum_dir flip_d( r_d[:,None] * (U0 + U1) ),
    r_d = rsqrt((s0 + s1)*256/2048 + eps).

v2 changes vs baseline:
  - fp8(e4m3) DoubleRow matmuls for in_proj/z/conv/out-proj/sumsq (weights
    pre-scaled x256 on host, descaled at the PSUM copy-out).
  - segsum decay matrices built on PE via selector-matmuls from the cumsum
    rows (l_i - l_j - 1e30*[i<j] accumulated in PSUM), single exp on ACT.
    Replaces per-head partition-broadcast DMAs + relu clamp.
  - dt_j and exp(T-l_j) state weights taken from column 127 of the decay
    matrix instead of a separate trep DMA + exp.
  - direct Silu activation for conv/z gates.
  - Dp*xs skip term accumulated into the Y PSUM as diagonal matmuls.
  - chunk-total decays broadcast to the state layout via PE selector matmul.
  - batched DMAs (x load 1/block, wc2 replication 4/block, u store 1/chunk).
"""
import numpy as np

L = 2048
DM = 1024  # d_model
Q = 128  # scan chunk
NCH = L // Q  # 16 chunks
BLK = 512
NBLK = L // BLK  # 4
CPB = BLK // Q  # 4 chunks per block
NH = 16  # local heads
P = 64  # head dim
NST = 16  # state dim
EPS = 1e-5
NEG = -1e30
WS = 1.0  # host weight prescale (bf16: none)
S1 = 1.0 / WS
CS = 1.0  # conv weight prescale (bf16: none)

_cache = {}


def _build_nc():
    import concourse.bass as bass
    import concourse.tile as tile
    import concourse.mybir as mybir
    from concourse import bacc
    from concourse.masks import make_identity
    from concourse.alu_op_type import AluOpType as alu

    f32 = mybir.dt.float32
    bf16 = mybir.dt.bfloat16
    f8 = mybir.dt.float8e4
    AF = mybir.ActivationFunctionType
    DR = mybir.MatmulPerfMode.DoubleRow

    nc = bacc.Bacc(trn_type="TRN2")

    # ---- DRAM I/O (per-core shapes; host prepares layouts) ----
    xt = nc.dram_tensor("xt", [128, 8, L], bf16, kind="ExternalInput")
    wt = nc.dram_tensor("wt", [128, 8, 2096], bf16, kind="ExternalInput")
    mft = nc.dram_tensor("mft", [128, 8, DM], bf16, kind="ExternalInput")
    cw = nc.dram_tensor("cw", [128, 36], f32, kind="ExternalInput")
    cb = nc.dram_tensor("cb", [128, 9], f32, kind="ExternalInput")
    hp = nc.dram_tensor("hp", [128, 10], f32, kind="ExternalInput")
    u = nc.dram_tensor("u", [L, DM], bf16, kind="ExternalOutput")
    s = nc.dram_tensor("s", [1, L], f32, kind="ExternalOutput")

    from contextlib import ExitStack

    with tile.TileContext(nc) as tc, ExitStack() as ctx:
        ep = ctx.enter_context
        const = ep(tc.tile_pool(name="const", bufs=1))
        statep = ep(tc.tile_pool(name="state", bufs=1))
        xtp = ep(tc.tile_pool(name="xtp", bufs=2))
        xsrp = ep(tc.tile_pool(name="xsrp", bufs=2))
        xssb = ep(tc.tile_pool(name="xssb", bufs=2))
        zsp = ep(tc.tile_pool(name="zsp", bufs=2))
        dtlp = ep(tc.tile_pool(name="dtlp", bufs=2))
        c2tp = ep(tc.tile_pool(name="c2tp", bufs=2))
        wrp = ep(tc.tile_pool(name="wrp", bufs=1))
        dscrp = ep(tc.tile_pool(name="dscrp", bufs=2, space="DRAM"))
        chkp = ep(tc.tile_pool(name="chkp", bufs=2))
        gpool = ep(tc.tile_pool(name="gp", bufs=2))
        ygp = ep(tc.tile_pool(name="ygp", bufs=2))
        usbp = ep(tc.tile_pool(name="usbp", bufs=2))
        y2blk = ep(tc.tile_pool(name="y2blk", bufs=1))
        pp_mm = ep(tc.tile_pool(name="pp_mm", bufs=2, space="PSUM"))
        pp_scr = ep(tc.tile_pool(name="pp_scr", bufs=3, space="PSUM"))
        pp_y = ep(tc.tile_pool(name="pp_y", bufs=2, space="PSUM"))
        pp_s = ep(tc.tile_pool(name="pp_s", bufs=1, space="PSUM"))
        if True:
            # ---------- constants / persistent ----------
            wt_sb = const.tile([128, 8, 2096], bf16)
            nc.sync.dma_start(wt_sb, wt[:, :, :])
            mft_sb = const.tile([128, 8, DM], bf16)
            nc.sync.dma_start(mft_sb, mft[:, :, :])
            cw_sb = const.tile([128, 36], f32)
            nc.sync.dma_start(cw_sb, cw[:, :])
            cb_sb = const.tile([128, 9], f32)
            nc.sync.dma_start(cb_sb, cb[:, :])
            hp_sb = const.tile([128, 10], f32)
            nc.sync.dma_start(hp_sb, hp[:, :])

            ident_b = const.tile([128, 128], bf16)
            make_identity(nc, ident_b)
            ident_f = const.tile([128, 128], f32)
            make_identity(nc, ident_f)
            # additive causal mask (station): utri[k, j] = NEG where k < j
            utri_b = const.tile([128, 128], bf16)
            nc.gpsimd.memset(utri_b, NEG)
            nc.gpsimd.affine_select(
                out=utri_b, in_=utri_b, compare_op=mybir.AluOpType.is_ge,
                fill=0.0, base=-1, pattern=[[1, 128]], channel_multiplier=-1,
            )
            # head-selector stations (rows 32..48): sel[32+m, h*128+j]=(m==h),
            # built as two affine_select equality half-planes on the 3D view.
            def eq_select(tile_ap, viewf, pattern, base, value=1.0):
                # keep where expr==0 via two is_ge selects (is_le unsupported)
                nc.gpsimd.memset(tile_ap, value)
                nc.gpsimd.affine_select(
                    out=viewf(tile_ap), in_=viewf(tile_ap),
                    compare_op=mybir.AluOpType.is_ge,
                    fill=0.0, base=base, pattern=pattern, channel_multiplier=-1,
                )
                negpat = [[-s, n] for s, n in pattern]
                nc.gpsimd.affine_select(
                    out=viewf(tile_ap), in_=viewf(tile_ap),
                    compare_op=mybir.AluOpType.is_ge,
                    fill=0.0, base=-base, pattern=negpat, channel_multiplier=1,
                )

            selbc = const.tile([48, NH * 128], bf16)
            negselbc = const.tile([48, NH * 128], bf16)
            eq_select(selbc, lambda t: t.rearrange("p (h j) -> p h j", j=128),
                      [[1, NH], [0, 128]], 32)
            eq_select(negselbc, lambda t: t.rearrange("p (h j) -> p h j", j=128),
                      [[1, NH], [0, 128]], 32, value=-1.0)
            # texp state-layout selector: seltx[32+m, t*128+p] = (m == 4t+p//32)
            seltx = const.tile([48, 4 * 128], f32)
            eq_select(seltx,
                      lambda t: t.rearrange("p (t k n) -> p t k n", k=4, n=32),
                      [[4, 4], [1, 4], [0, 32]], 32)
            onescol = const.tile([128, 1], bf16)
            nc.vector.memset(onescol, 1.0)
            onesq = const.tile([48, 128], f32)
            nc.vector.memset(onesq, 1.0)
            # conv diagonal weight tiles, built from cw columns
            convd = const.tile([128, 36, 128], bf16)
            for j in range(36):
                nc.scalar.mul(convd[:, j, :], ident_b, cw_sb[:, j : j + 1])
            # Dp diagonal stations per 128-channel group
            dpdiag = const.tile([128, 8, 128], bf16)
            for pr in range(8):
                nc.scalar.mul(dpdiag[:, pr, :], ident_b, hp_sb[:, 2 + pr : 3 + pr])

            # persistent scan state: 4 head-groups (heads 4t+k at partitions
            # 32k..32k+16), ping-pong A/B, bf16
            stA = statep.tile([128, 4, P], bf16, tag="stA")
            stB = statep.tile([128, 4, P], bf16, tag="stB")
            nc.vector.memset(stA, 0.0)
            nc.vector.memset(stB, 0.0)
            st = [stA, stB]
            # chunk-decay per-partition scalars, [state-tile, chunk]
            texp_st = statep.tile([128, 4, NCH], f32, tag="texp")
            nc.vector.memset(texp_st, 0.0)

            halo3 = statep.tile([128, 8, 3], bf16, tag="halo3")
            bch3 = statep.tile([32, 3], bf16, tag="bch3")

            for b in range(NBLK):
                bsl = slice(b * BLK, (b + 1) * BLK)
                # ---------- load x block (1 DMA) ----------
                xtb = xtp.tile([128, 8, BLK], bf16, tag="xtb")
                nc.sync.dma_start(xtb, xt[:, :, bsl])

                # ---------- in_proj (fp8 DoubleRow): xs tiles + BCdt ----------
                xsr = xsrp.tile([128, 8, BLK + 3], bf16, tag="xsr")
                bcr = xsrp.tile([32, BLK + 3], bf16, tag="bcr")
                dt_sp = dtlp.tile([48, BLK], f32, tag="dtsp")
                for et in range(9):
                    m = 128 if et < 8 else 48
                    ecol = et * 128 if et < 8 else 1024
                    ps = pp_mm.tile([128, BLK], f32, tag="mm")
                    for kt in range(8):
                        nc.tensor.matmul(
                            ps[:m, :], wt_sb[:, kt, ecol : ecol + m],
                            xtb[:, kt, :], start=(kt == 0), stop=(kt == 7),
                        )
                    if et < 8:
                        nc.scalar.activation(
                            xsr[:, et, 3 : 3 + BLK], ps, AF.Identity, scale=S1
                        )
                    else:
                        nc.scalar.activation(
                            bcr[:, 3 : 3 + BLK], ps[0:32, :], AF.Identity, scale=S1
                        )
                        nc.scalar.activation(
                            dt_sp[32:48, :], ps[32:48, :], AF.Identity, scale=S1
                        )
                # conv left halo from the previous block's tail
                if b == 0:
                    nc.vector.memset(xsr[:, :, 0:3], 0.0)
                    nc.vector.memset(bcr[:, 0:3], 0.0)
                else:
                    nc.vector.tensor_copy(xsr[:, :, 0:3], halo3)
                    nc.vector.tensor_copy(bcr[:, 0:3], bch3)
                if b < NBLK - 1:
                    nc.vector.tensor_copy(halo3, xsr[:, :, BLK : BLK + 3])
                    nc.vector.tensor_copy(bch3, bcr[:, BLK : BLK + 3])

                # ---------- conv (fp8 DR diag matmuls) + Silu ----------
                xs_sb = xssb.tile([128, 8, BLK], bf16, tag="xs")
                bct = dtlp.tile([32, BLK], bf16, tag="bct")
                ct4 = dtlp.tile([128, BLK], bf16, tag="ct4")
                for ct in range(9):
                    m = 128 if ct < 8 else 32
                    src_ = xsr[:, ct, :] if ct < 8 else bcr
                    ps = pp_mm.tile([128, BLK], f32, tag="mm")
                    for k in range(4):
                        nc.tensor.matmul(
                            ps[:m, :], convd[:m, ct * 4 + k, :m],
                            src_[:m, k : k + BLK],
                            start=(k == 0), stop=(k == 3),
                        )
                    dst = xs_sb[:, ct, :] if ct < 8 else bct
                    nc.scalar.activation(
                        dst, ps[:m, :], AF.Silu,
                        bias=cb_sb[:m, ct : ct + 1], scale=1.0 / CS,
                    )
                # C rows replicated to the four 32-aligned bases
                nc.gpsimd.memset(ct4, 0.0)
                for k4 in range(4):
                    nc.sync.dma_start(ct4[32 * k4 : 32 * k4 + 16, :], bct[16:32, :])

                # ---------- z in_proj (fp8 DR) + Silu ----------
                zs = zsp.tile([128, 8, BLK], bf16, tag="zs")
                for zt in range(8):
                    ps = pp_mm.tile([128, BLK], f32, tag="mm")
                    for kt in range(8):
                        nc.tensor.matmul(
                            ps, wt_sb[:, kt, 1072 + zt * 128 : 1200 + zt * 128],
                            xtb[:, kt, :], start=(kt == 0), stop=(kt == 7),
                        )
                    nc.scalar.activation(zs[:, zt, :], ps, AF.Silu, scale=S1)

                # ---------- dt pipeline (exp/ln table) ----------
                dtA = dtlp.tile([48, BLK], f32, tag="dtA")
                lcm = dtlp.tile([48, BLK], f32, tag="lcm")
                lcmb = dtlp.tile([48, BLK], bf16, tag="lcmb")
                wc2 = dtlp.tile([48, BLK], bf16, tag="wc2")
                nc.scalar.activation(
                    dt_sp[32:48, :], dt_sp[32:48, :], AF.Exp,
                    bias=hp_sb[32:48, 0:1],
                )
                nc.vector.tensor_scalar_add(dt_sp[32:48, :], dt_sp[32:48, :], 1.0)
                nc.scalar.activation(dt_sp[32:48, :], dt_sp[32:48, :], AF.Ln)
                nc.vector.tensor_scalar_mul(
                    dtA[32:48, :], dt_sp[32:48, :], hp_sb[32:48, 1:2]
                )
                for cc in range(CPB):
                    qs = slice(cc * Q, (cc + 1) * Q)
                    nc.vector.tensor_tensor_scan(
                        lcm[32:48, qs], onesq[32:48, :], dtA[32:48, qs],
                        0.0, alu.mult, alu.add,
                    )
                nc.scalar.activation(lcmb[32:48, :], lcm[32:48, :], AF.Identity)
                nc.scalar.activation(wc2[32:48, :], lcm[32:48, :], AF.Exp)
                # chunk total decays -> state layout via PE selector matmul
                texp_cm = dtlp.tile([48, CPB], f32, tag="texpcm")
                lv = lcm[32:48, :].rearrange("p (c q) -> p c q", q=Q)
                nc.scalar.activation(
                    texp_cm[32:48, :],
                    lv[:, :, 127:128].rearrange("p c one -> p (c one)"), AF.Exp)
                texps = pp_scr.tile([128, 512], f32, tag="scr")
                for t in range(4):
                    nc.tensor.matmul(
                        texps[:, t * CPB : (t + 1) * CPB],
                        seltx[32:48, t * 128 : (t + 1) * 128],
                        texp_cm[32:48, :], start=True, stop=True,
                    )
                nc.vector.tensor_copy(
                    texp_st[:, :, b * CPB : (b + 1) * CPB],
                    texps[:, 0:16].rearrange("p (t c) -> p t c", c=CPB),
                )
                # bounce wc2 through DRAM for the partition-replicated copy
                dscr2 = dscrp.tile([16, BLK], bf16, tag="dscr2")
                nc.sync.dma_start(dscr2, wc2[32:48, :])
                wc2rep = wrp.tile([128, 4, BLK], bf16, tag="wc2rep")
                nc.gpsimd.memset(wc2rep, 0.0)
                for k in range(4):
                    nc.sync.dma_start(
                        wc2rep[32 * k : 32 * k + 16, :, :],
                        bass.AP(dscr2.tensor, dscr2.offset + k * BLK,
                                [[0, 16], [4 * BLK, 4], [1, BLK]]),
                    )
                # C'' = C * exp(l_h) in state layout
                c2t = c2tp.tile([128, 4, BLK], bf16, tag="c2t")
                for t in range(4):
                    eng = nc.vector if t < 2 else nc.gpsimd
                    eng.tensor_tensor(
                        c2t[:, t, :], ct4, wc2rep[:, t, :], alu.mult
                    )

                # ---------- per-chunk scan ----------
                ssb = y2blk.tile([1, BLK], f32, tag="ssb")
                for cc in range(CPB):
                    c = b * CPB + cc
                    qs = slice(cc * Q, (cc + 1) * Q)
                    # xpos: PE-transpose xs chunk to position-major
                    xposr = chkp.tile([128, 8, Q], bf16, tag="xposr")
                    for w in range(2):
                        tp = pp_scr.tile([128, 512], f32, tag="scr")
                        tpb = tp.bitcast(bf16)
                        for ct in range(4):
                            nc.tensor.transpose(
                                tpb[:, ct * 128 : ct * 128 + 128],
                                xs_sb[:, w * 4 + ct, qs], ident_b,
                            )
                        if w == 0:
                            nc.scalar.copy(
                                xposr[:, 0:4, :],
                                tpb[:, 0:512].rearrange("p (a b) -> p a b", b=Q),
                            )
                        else:
                            nc.scalar.copy(
                                xposr[:, 4:8, :],
                                tpb[:, 0:512].rearrange("p (a b) -> p a b", b=Q),
                            )
                    # small transposes + S psums + g0 share one bank
                    sps = pp_s.tile([128, 512], f32, tag="sps")
                    nc.scalar.memzero(sps[:, 0:256])
                    nc.tensor.transpose(
                        sps[:, 256:272], dt_sp[32:48, qs], ident_f[32:48, 32:48]
                    )
                    spb = sps.bitcast(bf16)
                    nc.tensor.transpose(
                        spb[:, 544:560], bct[0:16, qs], ident_b[0:16, 0:16]
                    )
                    dtc = chkp.tile([128, NST], f32, tag="dtc")
                    bpos = chkp.tile([128, NST], bf16, tag="bpos")
                    nc.vector.tensor_copy(dtc, sps[:, 256:272])
                    nc.vector.tensor_copy(bpos, spb[:, 544:560])
                    # head-shared C@B^T -> g0[j, i] (unmasked)
                    nc.tensor.matmul(
                        sps[:, 384:512], bct[0:16, qs], ct4[0:16, qs],
                        start=True, stop=True,
                    )
                    g0b = chkp.tile([128, Q], bf16, tag="g0b")
                    nc.vector.tensor_copy(g0b, sps[:, 384:512])
                    # decay matrices on PE: D[j,(h,i)] = l_i - l_j + NEG*[i<j]
                    gdec = gpool.tile([128, NH, Q], bf16, tag="gdec")
                    for grp in range(4):
                        dps = pp_scr.tile([128, 512], f32, tag="scr")
                        for hh in range(4):
                            h = 4 * grp + hh
                            osl = slice(hh * 128, hh * 128 + 128)
                            nc.tensor.matmul(
                                dps[:, osl], utri_b, ident_b,
                                start=True, stop=False,
                            )
                            nc.tensor.matmul(
                                dps[:, osl],
                                selbc[32:48, h * 128 : h * 128 + 128],
                                lcmb[32:48, qs], start=False, stop=False,
                            )
                            nc.tensor.matmul(
                                dps[:, osl], lcmb[32:48, qs],
                                negselbc[32:48, h * 128 : h * 128 + 128],
                                start=False, stop=True,
                            )
                        hsl = slice(4 * grp, 4 * grp + 4)
                        nc.scalar.activation(
                            gdec[:, hsl, :],
                            dps.rearrange("p (h q) -> p h q", q=Q), AF.Exp,
                        )
                    # X scaled by dt_j (per position j, per head)
                    xpos = chkp.tile([128, NH, P], bf16, tag="xpos")
                    nc.vector.tensor_tensor(
                        xpos,
                        xposr.rearrange("p t c -> p (t c)")
                        .rearrange("p (h c) -> p h c", c=P),
                        dtc.rearrange("p (h one) -> p h one", one=1)
                        .to_broadcast([128, NH, P]),
                        alu.mult,
                    )
                    # state input weights from decay col 127: exp(T-l_j)
                    # (dt_j rides in via the dt-scaled xpos in the S matmul)
                    bws = chkp.tile([128, NH, NST], bf16, tag="bws")
                    nc.vector.tensor_tensor(
                        bws,
                        bpos.rearrange("p (one n) -> p one n", one=1)
                        .to_broadcast([128, NH, NST]),
                        gdec[:, :, 127:128].to_broadcast([128, NH, NST]),
                        alu.mult,
                    )
                    # in-place: gdec <- gdec * g0 (after bws consumed col 127)
                    for grp in range(4):
                        hsl = slice(4 * grp, 4 * grp + 4)
                        nc.vector.tensor_tensor(
                            gdec[:, hsl, :], gdec[:, hsl, :],
                            g0b.rearrange("p (one i) -> p one i", one=1)
                            .to_broadcast([128, 4, Q]),
                            alu.mult,
                        )
                    # Y psums: pairs 0-3 and 4-7
                    yA = pp_y.tile([128, 512], f32, tag="ytp")
                    yB = pp_y.tile([128, 512], f32, tag="ytp")
                    ypair = [yA, yB]
                    for h in range(NH):
                        k, t = h % 4, h // 4
                        pr = h // 2
                        # S^T = B_ws.T @ X_h
                        nc.tensor.matmul(
                            sps[32 * k : 32 * k + 16, t * 64 : t * 64 + 64],
                            bws[:, h, :], xpos[:, h, :],
                            start=True, stop=True, tile_position=(0, 32 * k),
                        )
                        # Y^T = X_h.T @ G^T (+ Dp diag + H^T.T @ C''^T)
                        yp = ypair[pr // 4]
                        r0 = 64 * (h % 2)
                        ysl = (slice(r0, r0 + 64),
                               slice((pr % 4) * 128, (pr % 4) * 128 + 128))
                        nc.tensor.matmul(
                            yp[ysl[0], ysl[1]], xpos[:, h, :], gdec[:, h, :],
                            start=True, stop=False,
                        )
                        nc.tensor.matmul(
                            yp[ysl[0], ysl[1]],
                            dpdiag[r0 : r0 + 64, pr, r0 : r0 + 64],
                            xs_sb[r0 : r0 + 64, pr, qs],
                            start=False, stop=(c == 0),
                        )
                        if c > 0:
                            nc.tensor.matmul(
                                yp[ysl[0], ysl[1]],
                                st[c % 2][32 * k : 32 * k + 16, t, :],
                                c2t[32 * k : 32 * k + 16, t, qs],
                                start=False, stop=True,
                                tile_position=(32 * k, r0),
                            )
                    # state recurrence
                    for t in range(4):
                        nc.vector.scalar_tensor_tensor(
                            st[(c + 1) % 2][:, t, :], st[c % 2][:, t, :],
                            texp_st[:, t, c : c + 1], sps[:, t * 64 : t * 64 + 64],
                            alu.mult, alu.add,
                        )
                    # ---------- gating ----------
                    ygf = ygp.tile([128, 8, Q], bf16, tag="ygf")
                    for w in range(2):
                        nc.vector.tensor_tensor(
                            ygf[:, w * 4 : w * 4 + 4, :],
                            ypair[w].rearrange("p (a b) -> p a b", b=Q),
                            zs[:, w * 4 : w * 4 + 4, qs], alu.mult,
                        )
                    y2c = chkp.tile([128, 8, Q], bf16, tag="y2c")
                    nc.scalar.activation(y2c, ygf, AF.Square, scale=1.0 / 16)
                    for ct in range(8):
                        nc.tensor.matmul(
                            sps[0:1, 280 : 280 + Q], onescol, y2c[:, ct, :],
                            start=(ct == 0), stop=(ct == 7),
                        )
                    nc.vector.tensor_copy(ssb[0:1, qs], sps[0:1, 280 : 280 + Q])
                    # ---------- U matmuls (fp8 DR) + store ----------
                    usb = usbp.tile([128, 2, BLK], bf16, tag="usb")
                    for oc in range(2):
                        ups = pp_mm.tile([128, BLK], f32, tag="mm")
                        for ct in range(8):
                            nc.tensor.matmul(
                                ups, ygf[:, ct, :],
                                mft_sb[:, ct, oc * 512 : oc * 512 + 512],
                                start=(ct == 0), stop=(ct == 7),
                            )
                        if oc == 0:
                            nc.vector.tensor_scalar_mul(usb[:, oc, :], ups, S1)
                        else:
                            nc.scalar.activation(
                                usb[:, oc, :], ups, AF.Identity, scale=S1
                            )
                    nc.gpsimd.dma_start(
                        u[c * 128 : c * 128 + 128, :],
                        usb.rearrange("p a b -> p (a b)"),
                    )
                nc.sync.dma_start(s[0:1, bsl], ssb)

    nc.finalize()
    return nc


def _get_nc():
    if "nc" not in _cache:
        _cache["nc"] = _build_nc()
    return _cache["nc"]


def _prep_core_inputs(inputs, b, d, hh):
    import ml_dtypes

    bf16 = ml_dtypes.bfloat16
    pre = "fwd" if d == 0 else "bwd"
    W = np.asarray(inputs[f"{pre}_in_proj_w"], np.float32)  # (4160, 1024)
    x = np.asarray(inputs["x"], np.float32)[b]  # (L, 1024)
    if d == 1:
        x = x[::-1]
    # x^T as (128, 8, L)
    xtv = np.ascontiguousarray(x.T.reshape(8, 128, L).transpose(1, 0, 2))
    # in_proj^T columns: [xs 1024 | B 16 | C 16 | dt 16 | z 1024]
    W_xs = W[2048 + hh * 1024 : 3072 + hh * 1024]
    W_B = W[4096:4112]
    W_C = W[4112:4128]
    W_dt = W[4128 + hh * 16 : 4144 + hh * 16]
    W_z = W[hh * 1024 : 1024 + hh * 1024]
    Wt = np.concatenate([W_xs, W_B, W_C, W_dt, W_z], axis=0).T * WS  # (1024, 2096)
    wtv = np.ascontiguousarray(Wt.reshape(8, 128, 2096).transpose(1, 0, 2))
    # fused output projection
    Wo = np.asarray(inputs[f"{pre}_out_proj_w"], np.float32)  # (1024, 2048)
    Wl = np.asarray(inputs["layer_out_proj_w"], np.float32)  # (1024, 2048)
    nw = np.asarray(inputs[f"{pre}_norm_w"], np.float32)
    ch = slice(hh * 1024, hh * 1024 + 1024)
    M = (Wl[:, d * 1024 : d * 1024 + 1024] @ Wo)[:, ch] * nw[ch][None, :]
    MfT = M.T * WS  # (1024 c, 1024 o)
    mftv = np.ascontiguousarray(MfT.reshape(8, 128, 1024).transpose(1, 0, 2))
    # conv: diag values per (c-tile, tap) column
    cwf = np.asarray(inputs[f"{pre}_conv_w"], np.float32)[:, 0, :]  # (2080, 4)
    cwl = np.concatenate([cwf[hh * 1024 : 1024 + hh * 1024], cwf[2048:2080]], axis=0)
    cwv = np.zeros((128, 36), np.float32)
    for ct in range(9):
        n = 128 if ct < 8 else 32
        for k in range(4):
            cwv[:n, ct * 4 + k] = cwl[ct * 128 : ct * 128 + n, k] * CS
    cbf = np.asarray(inputs[f"{pre}_conv_b"], np.float32)
    cbl = np.concatenate([cbf[hh * 1024 : 1024 + hh * 1024], cbf[2048:2080]])
    cbv = np.zeros((128, 9), np.float32)
    for ct in range(9):
        n = 128 if ct < 8 else 32
        cbv[:n, ct] = cbl[ct * 128 : ct * 128 + n]
    # host params
    hpv = np.zeros((128, 10), np.float32)
    hs = slice(hh * 16, hh * 16 + 16)
    hpv[32:48, 0] = np.asarray(inputs[f"{pre}_dt_bias"], np.float32)[hs]
    hpv[32:48, 1] = -np.exp(np.asarray(inputs[f"{pre}_A_log"], np.float32)[hs])
    Dp = np.asarray(inputs[f"{pre}_Dp"], np.float32)[hs]
    for pr in range(8):
        rows = (np.arange(128) + pr * 128) // 64  # local head of channel
        hpv[:, 2 + pr] = Dp[rows]
    return {
        "xt": xtv.astype(bf16),
        "wt": wtv.astype(bf16),
        "mft": mftv.astype(bf16),
        "cw": cwv,
        "cb": cbv,
        "hp": hpv,
    }


def _combine(inputs, results):
    x = np.asarray(inputs["x"], np.float32)
    scale = np.asarray(inputs["layer_scale"], np.float32)
    out = x.copy()
    i = 0
    for b in range(2):
        for d in range(2):
            U0 = np.asarray(results[i]["u"], np.float32)
            s0 = np.asarray(results[i]["s"][0], np.float32)
            U1 = np.asarray(results[i + 1]["u"], np.float32)
            s1 = np.asarray(results[i + 1]["s"][0], np.float32)
            i += 2
            r = 1.0 / np.sqrt((s0 + s1) * 256.0 / 2048.0 + EPS)
            contrib = r[:, None] * (U0 + U1)
            if d == 1:
                contrib = contrib[::-1]
            out[b] += contrib * scale[None, :]
    return out


def _run(inputs, trace=False, core_ids=None):
    from concourse.bass_utils import run_bass_kernel_spmd

    nc = _get_nc()
    in_maps = []
    for b in range(2):
        for d in range(2):
            for hh in range(2):
                in_maps.append(_prep_core_inputs(inputs, b, d, hh))
    if core_ids is None:
        core_ids = list(range(8))
    res = run_bass_kernel_spmd(
        nc, in_maps[: len(core_ids)], core_ids=core_ids, trace=trace
    )
    return res


def kernel(**inputs):
    res = _run(inputs)
    return _combine(inputs, res.results)


# revision 26
# speedup vs baseline: 1.2033x; 1.2033x over previous
"""BiMambaLayer Trainium2 kernel (v2).

Sharding: 8 cores = batch(2) x direction(2) x head-half(2). Each core runs the
full L=2048 sequence of one (batch, direction) through 16 of the 32 heads of
that direction's Mamba2 block, plus the fused output projection restricted to
its 1024 d_inner channels. The gated-RMSNorm row scale commutes with the output
projections, so each core returns an unnormalized partial projection U and a
partial sum-of-squares s; the host combines:
    out[b] = x[b] + scale * sum_dir flip_d( r_d[:,None] * (U0 + U1) ),
    r_d = rsqrt((s0 + s1)*256/2048 + eps).

v2 changes vs baseline:
  - fp8(e4m3) DoubleRow matmuls for in_proj/z/conv/out-proj/sumsq (weights
    pre-scaled x256 on host, descaled at the PSUM copy-out).
  - segsum decay matrices built on PE via selector-matmuls from the cumsum
    rows (l_i - l_j - 1e30*[i<j] accumulated in PSUM), single exp on ACT.
    Replaces per-head partition-broadcast DMAs + relu clamp.
  - dt_j and exp(T-l_j) state weights taken from column 127 of the decay
    matrix instead of a separate trep DMA + exp.
  - direct Silu activation for conv/z gates.
  - Dp*xs skip term accumulated into the Y PSUM as diagonal matmuls.
  - chunk-total decays broadcast to the state layout via PE selector matmul.
  - batched DMAs (x load 1/block, wc2 replication 4/block, u store 1/chunk).
"""
import numpy as np

L = 2048
DM = 1024  # d_model
Q = 128  # scan chunk
NCH = L // Q  # 16 chunks
BLK = 512
NBLK = L // BLK  # 4
CPB = BLK // Q  # 4 chunks per block
NH = 16  # local heads
P = 64  # head dim
NST = 16  # state dim
EPS = 1e-5
NEG = -1e30
WS = 1.0  # host weight prescale (bf16: none)
S1 = 1.0 / WS
CS = 1.0  # conv weight prescale (bf16: none)

_cache = {}


def _build_nc():
    import concourse.bass as bass
    import concourse.tile as tile
    import concourse.mybir as mybir
    from concourse import bacc
    from concourse.masks import make_identity
    from concourse.alu_op_type import AluOpType as alu

    f32 = mybir.dt.float32
    bf16 = mybir.dt.bfloat16
    f8 = mybir.dt.float8e4
    AF = mybir.ActivationFunctionType
    DR = mybir.MatmulPerfMode.DoubleRow

    nc = bacc.Bacc(trn_type="TRN2")

    # ---- DRAM I/O (per-core shapes; host prepares layouts) ----
    xt = nc.dram_tensor("xt", [128, 8, L], bf16, kind="ExternalInput")
    wt = nc.dram_tensor("wt", [128, 8, 2096], bf16, kind="ExternalInput")
    mft = nc.dram_tensor("mft", [128, 8, DM], bf16, kind="ExternalInput")
    cw = nc.dram_tensor("cw", [128, 36], f32, kind="ExternalInput")
    cb = nc.dram_tensor("cb", [128, 18], f32, kind="ExternalInput")
    hp = nc.dram_tensor("hp", [128, 10], f32, kind="ExternalInput")
    u = nc.dram_tensor("u", [L, DM], bf16, kind="ExternalOutput")
    s = nc.dram_tensor("s", [1, L], f32, kind="ExternalOutput")

    from contextlib import ExitStack

    with tile.TileContext(nc) as tc, ExitStack() as ctx:
        ep = ctx.enter_context
        const = ep(tc.tile_pool(name="const", bufs=1))
        statep = ep(tc.tile_pool(name="state", bufs=1))
        xtp = ep(tc.tile_pool(name="xtp", bufs=2))
        xsrp = ep(tc.tile_pool(name="xsrp", bufs=2))
        xssb = ep(tc.tile_pool(name="xssb", bufs=2))
        zsp = ep(tc.tile_pool(name="zsp", bufs=2))
        dtlp = ep(tc.tile_pool(name="dtlp", bufs=2))
        c2tp = ep(tc.tile_pool(name="c2tp", bufs=2))
        wrp = ep(tc.tile_pool(name="wrp", bufs=1))
        dscrp = ep(tc.tile_pool(name="dscrp", bufs=2, space="DRAM"))
        chkp = ep(tc.tile_pool(name="chkp", bufs=2))
        gpool = ep(tc.tile_pool(name="gp", bufs=2))
        ygp = ep(tc.tile_pool(name="ygp", bufs=2))
        usbp = ep(tc.tile_pool(name="usbp", bufs=2))
        y2blk = ep(tc.tile_pool(name="y2blk", bufs=1))
        pp_mm = ep(tc.tile_pool(name="pp_mm", bufs=2, space="PSUM"))
        pp_scr = ep(tc.tile_pool(name="pp_scr", bufs=3, space="PSUM"))
        pp_y = ep(tc.tile_pool(name="pp_y", bufs=2, space="PSUM"))
        pp_s = ep(tc.tile_pool(name="pp_s", bufs=1, space="PSUM"))
        if True:
            # ---------- constants / persistent ----------
            wt_sb = const.tile([128, 8, 2096], bf16)
            nc.sync.dma_start(wt_sb, wt[:, :, :])
            mft_sb = const.tile([128, 8, DM], bf16)
            nc.sync.dma_start(mft_sb, mft[:, :, :])
            cw_sb = const.tile([128, 36], f32)
            nc.sync.dma_start(cw_sb, cw[:, :])
            cb_sb = const.tile([128, 18], f32)
            nc.sync.dma_start(cb_sb, cb[:, :])
            hp_sb = const.tile([128, 10], f32)
            nc.sync.dma_start(hp_sb, hp[:, :])

            ident_b = const.tile([128, 128], bf16)
            make_identity(nc, ident_b)
            ident_f = const.tile([128, 128], f32)
            make_identity(nc, ident_f)
            # additive causal mask (station): utri[k, j] = NEG where k < j
            utri_b = const.tile([128, 128], bf16)
            nc.gpsimd.memset(utri_b, NEG)
            nc.gpsimd.affine_select(
                out=utri_b, in_=utri_b, compare_op=mybir.AluOpType.is_ge,
                fill=0.0, base=-1, pattern=[[1, 128]], channel_multiplier=-1,
            )
            # head-selector stations (rows 32..48): sel[32+m, h*128+j]=(m==h),
            # built as two affine_select equality half-planes on the 3D view.
            def eq_select(tile_ap, viewf, pattern, base, value=1.0):
                # keep where expr==0 via two is_ge selects (is_le unsupported)
                nc.gpsimd.memset(tile_ap, value)
                nc.gpsimd.affine_select(
                    out=viewf(tile_ap), in_=viewf(tile_ap),
                    compare_op=mybir.AluOpType.is_ge,
                    fill=0.0, base=base, pattern=pattern, channel_multiplier=-1,
                )
                negpat = [[-s, n] for s, n in pattern]
                nc.gpsimd.affine_select(
                    out=viewf(tile_ap), in_=viewf(tile_ap),
                    compare_op=mybir.AluOpType.is_ge,
                    fill=0.0, base=-base, pattern=negpat, channel_multiplier=1,
                )

            selbc = const.tile([48, NH * 128], bf16)
            negselbc = const.tile([48, NH * 128], bf16)
            eq_select(selbc, lambda t: t.rearrange("p (h j) -> p h j", j=128),
                      [[1, NH], [0, 128]], 32)
            eq_select(negselbc, lambda t: t.rearrange("p (h j) -> p h j", j=128),
                      [[1, NH], [0, 128]], 32, value=-1.0)
            # texp state-layout selector: seltx[32+m, t*128+p] = (m == 4t+p//32)
            seltx = const.tile([48, 4 * 128], bf16)
            eq_select(seltx,
                      lambda t: t.rearrange("p (t k n) -> p t k n", k=4, n=32),
                      [[4, 4], [1, 4], [0, 32]], 32)
            idq = const.tile([128, 4, 128], bf16)
            for q in range(4):
                nc.vector.tensor_copy(idq[:, q, :], ident_b)
            onescol = const.tile([128, 1], bf16)
            nc.vector.memset(onescol, 1.0)
            onesq = const.tile([48, 128], f32)
            nc.vector.memset(onesq, 1.0)
            # conv diagonal weight tiles, built from cw columns
            convd = const.tile([128, 36, 128], bf16)
            for j in range(36):
                nc.scalar.mul(convd[:, j, :], ident_b, cw_sb[:, j : j + 1])
            # Dp diagonal stations per 128-channel group
            dpdiag = const.tile([128, 8, 128], bf16)
            for pr in range(8):
                nc.scalar.mul(dpdiag[:, pr, :], ident_b, hp_sb[:, 2 + pr : 3 + pr])

            # persistent scan state: 4 head-groups (heads 4t+k at partitions
            # 32k..32k+16), ping-pong A/B, bf16
            stA = statep.tile([128, 4, P], bf16, tag="stA")
            stB = statep.tile([128, 4, P], bf16, tag="stB")
            nc.vector.memset(stA, 0.0)
            nc.vector.memset(stB, 0.0)
            st = [stA, stB]
            # chunk-decay per-partition scalars, [state-tile, chunk]
            texp_st = statep.tile([128, 4, NCH], f32, tag="texp")
            nc.vector.memset(texp_st, 0.0)

            halo3 = statep.tile([128, 8, 3], bf16, tag="halo3")
            bch3 = statep.tile([32, 3], bf16, tag="bch3")

            for b in range(NBLK):
                bsl = slice(b * BLK, (b + 1) * BLK)
                # ---------- load x block (1 DMA) ----------
                xtb = xtp.tile([128, 8, BLK], bf16, tag="xtb")
                nc.sync.dma_start(xtb, xt[:, :, bsl])

                # ---------- in_proj (fp8 DoubleRow): xs tiles + BCdt ----------
                xsr = xsrp.tile([128, 8, BLK + 3], bf16, tag="xsr")
                bcr = xsrp.tile([32, BLK + 3], bf16, tag="bcr")
                dt_sp = dtlp.tile([48, BLK], f32, tag="dtsp")
                for et in range(9):
                    m = 128 if et < 8 else 48
                    ecol = et * 128 if et < 8 else 1024
                    ps = pp_mm.tile([128, BLK], f32, tag="mm")
                    for kt in range(8):
                        nc.tensor.matmul(
                            ps[:m, :], wt_sb[:, kt, ecol : ecol + m],
                            xtb[:, kt, :], start=(kt == 0), stop=(kt == 7),
                        )
                    if et < 8:
                        nc.scalar.activation(
                            xsr[:, et, 3 : 3 + BLK], ps, AF.Identity, scale=S1
                        )
                    else:
                        nc.scalar.activation(
                            bcr[:, 3 : 3 + BLK], ps[0:32, :], AF.Identity, scale=S1
                        )
                        nc.scalar.activation(
                            dt_sp[32:48, :], ps[32:48, :], AF.Identity, scale=S1
                        )
                # conv left halo from the previous block's tail
                if b == 0:
                    nc.vector.memset(xsr[:, :, 0:3], 0.0)
                    nc.vector.memset(bcr[:, 0:3], 0.0)
                else:
                    nc.vector.tensor_copy(xsr[:, :, 0:3], halo3)
                    nc.vector.tensor_copy(bcr[:, 0:3], bch3)
                if b < NBLK - 1:
                    nc.vector.tensor_copy(halo3, xsr[:, :, BLK : BLK + 3])
                    nc.vector.tensor_copy(bch3, bcr[:, BLK : BLK + 3])

                # ---------- conv (fp8 DR diag matmuls) + Silu ----------
                xs_sb = xssb.tile([128, 8, BLK], bf16, tag="xs")
                bct = dtlp.tile([32, BLK], bf16, tag="bct")
                ct4 = dtlp.tile([128, BLK], bf16, tag="ct4")
                for ct in range(9):
                    m = 128 if ct < 8 else 32
                    src_ = xsr[:, ct, :] if ct < 8 else bcr
                    ps = pp_mm.tile([128, BLK], f32, tag="mm")
                    for k in range(4):
                        nc.tensor.matmul(
                            ps[:m, :], convd[:m, ct * 4 + k, :m],
                            src_[:m, k : k + BLK],
                            start=(k == 0), stop=(k == 3),
                        )
                    dst = xs_sb[:, ct, :] if ct < 8 else bct
                    xu = ygp.tile([128, BLK], bf16, tag="xu")
                    nc.scalar.activation(
                        xu[:m, :], ps[:m, :], AF.Identity,
                        bias=cb_sb[:m, ct : ct + 1],
                    )
                    th = ygp.tile([128, BLK], bf16, tag="th")
                    nc.scalar.activation(
                        th[:m, :], ps[:m, :], AF.Tanh,
                        bias=cb_sb[:m, 9 + ct : 10 + ct], scale=0.5,
                    )
                    nc.vector.scalar_tensor_tensor(
                        dst, th[:m, :], 1.0, xu[:m, :], alu.add, alu.mult
                    )
                # C rows replicated to the four 32-aligned bases
                nc.gpsimd.memset(ct4, 0.0)
                for k4 in range(4):
                    nc.sync.dma_start(ct4[32 * k4 : 32 * k4 + 16, :], bct[16:32, :])

                # ---------- z in_proj (fp8 DR) + Silu ----------
                zs = zsp.tile([128, 8, BLK], bf16, tag="zs")
                for zt in range(8):
                    ps = pp_mm.tile([128, BLK], f32, tag="mm")
                    for kt in range(8):
                        nc.tensor.matmul(
                            ps, wt_sb[:, kt, 1072 + zt * 128 : 1200 + zt * 128],
                            xtb[:, kt, :], start=(kt == 0), stop=(kt == 7),
                        )
                    zt_t = ygp.tile([128, BLK], bf16, tag="th")
                    nc.scalar.activation(zt_t, ps, AF.Tanh, scale=0.5)
                    nc.vector.tensor_scalar(zt_t, zt_t, 0.5, 0.5, alu.mult, alu.add)
                    nc.vector.tensor_tensor(zs[:, zt, :], zt_t, ps, alu.mult)

                # ---------- dt pipeline (exp/ln table) ----------
                dtA = dtlp.tile([48, BLK], f32, tag="dtA")
                lcm = dtlp.tile([48, BLK], f32, tag="lcm")
                lcmb = dtlp.tile([48, BLK], bf16, tag="lcmb")
                wc2 = dtlp.tile([48, BLK], bf16, tag="wc2")
                nc.scalar.activation(
                    dt_sp[32:48, :], dt_sp[32:48, :], AF.Exp,
                    bias=hp_sb[32:48, 0:1],
                )
                nc.vector.tensor_scalar_add(dt_sp[32:48, :], dt_sp[32:48, :], 1.0)
                nc.scalar.activation(dt_sp[32:48, :], dt_sp[32:48, :], AF.Ln)
                nc.vector.tensor_scalar_mul(
                    dtA[32:48, :], dt_sp[32:48, :], hp_sb[32:48, 1:2]
                )
                for cc in range(CPB):
                    qs = slice(cc * Q, (cc + 1) * Q)
                    nc.vector.tensor_tensor_scan(
                        lcm[32:48, qs], onesq[32:48, :], dtA[32:48, qs],
                        0.0, alu.mult, alu.add,
                    )
                nc.scalar.activation(lcmb[32:48, :], lcm[32:48, :], AF.Identity)
                nc.scalar.activation(wc2[32:48, :], lcm[32:48, :], AF.Exp)
                # chunk total decays -> state layout via PE selector matmul
                texp_cm = dtlp.tile([48, CPB], bf16, tag="texpcm")
                lv = lcm[32:48, :].rearrange("p (c q) -> p c q", q=Q)
                nc.scalar.activation(
                    texp_cm[32:48, :],
                    lv[:, :, 127:128].rearrange("p c one -> p (c one)"), AF.Exp)
                texps = pp_scr.tile([128, 512], f32, tag="scr")
                for t in range(4):
                    nc.tensor.matmul(
                        texps[:, t * CPB : (t + 1) * CPB],
                        seltx[32:48, t * 128 : (t + 1) * 128],
                        texp_cm[32:48, :], start=True, stop=True,
                    )
                nc.vector.tensor_copy(
                    texp_st[:, :, b * CPB : (b + 1) * CPB],
                    texps[:, 0:16].rearrange("p (t c) -> p t c", c=CPB),
                )
                # bounce wc2 through DRAM for the partition-replicated copy
                dscr2 = dscrp.tile([16, BLK], bf16, tag="dscr2")
                nc.sync.dma_start(dscr2, wc2[32:48, :])
                wc2rep = wrp.tile([128, 4, BLK], bf16, tag="wc2rep")
                nc.gpsimd.memset(wc2rep, 0.0)
                for k in range(4):
                    nc.sync.dma_start(
                        wc2rep[32 * k : 32 * k + 16, :, :],
                        bass.AP(dscr2.tensor, dscr2.offset + k * BLK,
                                [[0, 16], [4 * BLK, 4], [1, BLK]]),
                    )
                # C'' = C * exp(l_h) in state layout
                c2t = c2tp.tile([128, 4, BLK], bf16, tag="c2t")
                for t in range(4):
                    eng = nc.vector if t < 2 else nc.gpsimd
                    eng.tensor_tensor(
                        c2t[:, t, :], ct4, wc2rep[:, t, :], alu.mult
                    )

                # ---------- per-chunk scan ----------
                ssb = y2blk.tile([1, BLK], f32, tag="ssb")
                for cc in range(CPB):
                    c = b * CPB + cc
                    qs = slice(cc * Q, (cc + 1) * Q)
                    # xpos: PE-transpose xs chunk to position-major
                    xposr = chkp.tile([128, 8, Q], bf16, tag="xposr")
                    for w in range(2):
                        tp = pp_scr.tile([128, 512], f32, tag="scr")
                        tpb = tp.bitcast(bf16)
                        for ct in range(4):
                            nc.tensor.transpose(
                                tpb[:, ct * 128 : ct * 128 + 128],
                                xs_sb[:, w * 4 + ct, qs], ident_b,
                            )
                        if w == 0:
                            nc.scalar.copy(
                                xposr[:, 0:4, :],
                                tpb[:, 0:512].rearrange("p (a b) -> p a b", b=Q),
                            )
                        else:
                            nc.scalar.copy(
                                xposr[:, 4:8, :],
                                tpb[:, 0:512].rearrange("p (a b) -> p a b", b=Q),
                            )
                    # small transposes + S psums + g0 share one bank
                    sps = pp_s.tile([128, 512], f32, tag="sps")
                    nc.scalar.memzero(sps[:, 0:256])
                    nc.tensor.transpose(
                        sps[:, 256:272], dt_sp[32:48, qs], ident_f[32:48, 32:48]
                    )
                    spb = sps.bitcast(bf16)
                    nc.tensor.transpose(
                        spb[:, 544:560], bct[0:16, qs], ident_b[0:16, 0:16]
                    )
                    dtc = chkp.tile([128, NST], f32, tag="dtc")
                    bpos = chkp.tile([128, NST], bf16, tag="bpos")
                    nc.vector.tensor_copy(dtc, sps[:, 256:272])
                    nc.vector.tensor_copy(bpos, spb[:, 544:560])
                    # head-shared C@B^T -> g0[j, i] (unmasked)
                    nc.tensor.matmul(
                        sps[:, 384:512], bct[0:16, qs], ct4[0:16, qs],
                        start=True, stop=True,
                    )
                    g0b = chkp.tile([128, Q], bf16, tag="g0b")
                    nc.vector.tensor_copy(g0b, sps[:, 384:512])
                    # decay matrices on PE: D[j,(h,i)] = l_i - l_j + NEG*[i<j]
                    gdec = gpool.tile([128, NH, Q], bf16, tag="gdec")
                    for grp in range(4):
                        dps = pp_scr.tile([128, 512], f32, tag="scr")
                        # mask: -1e30 where i<j, replicated for 4 heads
                        nc.tensor.matmul(
                            dps, utri_b, idq.rearrange("p a b -> p (a b)"),
                            start=True, stop=False, skip_group_check=True,
                        )
                        # -l_h[j]: one 512-wide matmul per group
                        nc.tensor.matmul(
                            dps, lcmb[32:48, qs],
                            negselbc[32:48, grp * 512 : grp * 512 + 512],
                            start=False, stop=False, skip_group_check=True,
                        )
                        for hh in range(4):
                            h = 4 * grp + hh
                            osl = slice(hh * 128, hh * 128 + 128)
                            nc.tensor.matmul(
                                dps[:, osl],
                                selbc[32:48, h * 128 : h * 128 + 128],
                                lcmb[32:48, qs], start=False, stop=(hh == 3),
                                skip_group_check=True,
                            )
                        hsl = slice(4 * grp, 4 * grp + 4)
                        nc.scalar.activation(
                            gdec[:, hsl, :],
                            dps.rearrange("p (h q) -> p h q", q=Q), AF.Exp,
                        )
                    # X scaled by dt_j (per position j, per head)
                    xpos = chkp.tile([128, NH, P], bf16, tag="xpos")
                    nc.vector.tensor_tensor(
                        xpos,
                        xposr.rearrange("p t c -> p (t c)")
                        .rearrange("p (h c) -> p h c", c=P),
                        dtc.rearrange("p (h one) -> p h one", one=1)
                        .to_broadcast([128, NH, P]),
                        alu.mult,
                    )
                    # state input weights from decay col 127: exp(T-l_j)
                    # (dt_j rides in via the dt-scaled xpos in the S matmul)
                    bws = chkp.tile([128, NH, NST], bf16, tag="bws")
                    nc.vector.tensor_tensor(
                        bws,
                        bpos.rearrange("p (one n) -> p one n", one=1)
                        .to_broadcast([128, NH, NST]),
                        gdec[:, :, 127:128].to_broadcast([128, NH, NST]),
                        alu.mult,
                    )
                    # in-place: gdec <- gdec * g0 (after bws consumed col 127)
                    for grp in range(4):
                        hsl = slice(4 * grp, 4 * grp + 4)
                        nc.vector.tensor_tensor(
                            gdec[:, hsl, :], gdec[:, hsl, :],
                            g0b.rearrange("p (one i) -> p one i", one=1)
                            .to_broadcast([128, 4, Q]),
                            alu.mult,
                        )
                    # Y psums: pairs 0-3 and 4-7
                    yA = pp_y.tile([128, 512], f32, tag="ytp")
                    yB = pp_y.tile([128, 512], f32, tag="ytp")
                    ypair = [yA, yB]
                    for h in range(NH):
                        k, t = h % 4, h // 4
                        pr = h // 2
                        # S^T = B_ws.T @ X_h
                        nc.tensor.matmul(
                            sps[32 * k : 32 * k + 16, t * 64 : t * 64 + 64],
                            bws[:, h, :], xpos[:, h, :],
                            start=True, stop=True, tile_position=(0, 32 * k),
                        )
                        # Y^T = X_h.T @ G^T (+ Dp diag + H^T.T @ C''^T)
                        yp = ypair[pr // 4]
                        r0 = 64 * (h % 2)
                        ysl = (slice(r0, r0 + 64),
                               slice((pr % 4) * 128, (pr % 4) * 128 + 128))
                        nc.tensor.matmul(
                            yp[ysl[0], ysl[1]], xpos[:, h, :], gdec[:, h, :],
                            start=True, stop=False, skip_group_check=True,
                        )
                        if h % 2 == 1:
                            nc.tensor.matmul(
                                yp[:, ysl[1]], dpdiag[:, pr, :],
                                xs_sb[:, pr, qs],
                                start=False, stop=(c == 0),
                                skip_group_check=True,
                            )
                        if c > 0:
                            nc.tensor.matmul(
                                yp[ysl[0], ysl[1]],
                                st[c % 2][32 * k : 32 * k + 16, t, :],
                                c2t[32 * k : 32 * k + 16, t, qs],
                                start=False, stop=(h % 2 == 1),
                                tile_position=(32 * k, r0),
                                skip_group_check=True,
                            )
                    # state recurrence
                    for t in range(4):
                        nc.vector.scalar_tensor_tensor(
                            st[(c + 1) % 2][:, t, :], st[c % 2][:, t, :],
                            texp_st[:, t, c : c + 1], sps[:, t * 64 : t * 64 + 64],
                            alu.mult, alu.add,
                        )
                    # ---------- gating ----------
                    ygf = ygp.tile([128, 8, Q], bf16, tag="ygf")
                    for w in range(2):
                        nc.vector.tensor_tensor(
                            ygf[:, w * 4 : w * 4 + 4, :],
                            ypair[w].rearrange("p (a b) -> p a b", b=Q),
                            zs[:, w * 4 : w * 4 + 4, qs], alu.mult,
                        )
                    y2c = chkp.tile([128, 8, Q], bf16, tag="y2c")
                    nc.scalar.activation(y2c, ygf, AF.Square, scale=1.0 / 16)
                    for ct in range(8):
                        nc.tensor.matmul(
                            sps[0:1, 280 : 280 + Q], onescol, y2c[:, ct, :],
                            start=(ct == 0), stop=(ct == 7),
                        )
                    nc.vector.tensor_copy(ssb[0:1, qs], sps[0:1, 280 : 280 + Q])
                    # ---------- U matmuls (fp8 DR) + store ----------
                    usb = usbp.tile([128, 2, BLK], bf16, tag="usb")
                    for oc in range(2):
                        ups = pp_mm.tile([128, BLK], f32, tag="mm")
                        for ct in range(8):
                            nc.tensor.matmul(
                                ups, ygf[:, ct, :],
                                mft_sb[:, ct, oc * 512 : oc * 512 + 512],
                                start=(ct == 0), stop=(ct == 7),
                            )
                        if oc == 0:
                            nc.vector.tensor_scalar_mul(usb[:, oc, :], ups, S1)
                        else:
                            nc.scalar.activation(
                                usb[:, oc, :], ups, AF.Identity, scale=S1
                            )
                    nc.gpsimd.dma_start(
                        u[c * 128 : c * 128 + 128, :],
                        usb.rearrange("p a b -> p (a b)"),
                    )
                nc.sync.dma_start(s[0:1, bsl], ssb)

    nc.finalize()
    return nc


def _get_nc():
    if "nc" not in _cache:
        _cache["nc"] = _build_nc()
    return _cache["nc"]


def _prep_core_inputs(inputs, b, d, hh):
    import ml_dtypes

    bf16 = ml_dtypes.bfloat16
    pre = "fwd" if d == 0 else "bwd"
    W = np.asarray(inputs[f"{pre}_in_proj_w"], np.float32)  # (4160, 1024)
    x = np.asarray(inputs["x"], np.float32)[b]  # (L, 1024)
    if d == 1:
        x = x[::-1]
    # x^T as (128, 8, L)
    xtv = np.ascontiguousarray(x.T.reshape(8, 128, L).transpose(1, 0, 2))
    # in_proj^T columns: [xs 1024 | B 16 | C 16 | dt 16 | z 1024]
    W_xs = W[2048 + hh * 1024 : 3072 + hh * 1024]
    W_B = W[4096:4112]
    W_C = W[4112:4128]
    W_dt = W[4128 + hh * 16 : 4144 + hh * 16]
    W_z = W[hh * 1024 : 1024 + hh * 1024]
    Wt = np.concatenate([W_xs, W_B, W_C, W_dt, W_z], axis=0).T * WS  # (1024, 2096)
    wtv = np.ascontiguousarray(Wt.reshape(8, 128, 2096).transpose(1, 0, 2))
    # fused output projection
    Wo = np.asarray(inputs[f"{pre}_out_proj_w"], np.float32)  # (1024, 2048)
    Wl = np.asarray(inputs["layer_out_proj_w"], np.float32)  # (1024, 2048)
    nw = np.asarray(inputs[f"{pre}_norm_w"], np.float32)
    ch = slice(hh * 1024, hh * 1024 + 1024)
    M = (Wl[:, d * 1024 : d * 1024 + 1024] @ Wo)[:, ch] * nw[ch][None, :]
    M = M * 0.125  # xs/B/C carry 2x from the fused silu; y_g is 8x
    MfT = M.T * WS  # (1024 c, 1024 o)
    mftv = np.ascontiguousarray(MfT.reshape(8, 128, 1024).transpose(1, 0, 2))
    # conv: diag values per (c-tile, tap) column
    cwf = np.asarray(inputs[f"{pre}_conv_w"], np.float32)[:, 0, :]  # (2080, 4)
    cwl = np.concatenate([cwf[hh * 1024 : 1024 + hh * 1024], cwf[2048:2080]], axis=0)
    cwv = np.zeros((128, 36), np.float32)
    for ct in range(9):
        n = 128 if ct < 8 else 32
        for k in range(4):
            cwv[:n, ct * 4 + k] = cwl[ct * 128 : ct * 128 + n, k] * CS
    cbf = np.asarray(inputs[f"{pre}_conv_b"], np.float32)
    cbl = np.concatenate([cbf[hh * 1024 : 1024 + hh * 1024], cbf[2048:2080]])
    cbv = np.zeros((128, 18), np.float32)
    for ct in range(9):
        n = 128 if ct < 8 else 32
        cbv[:n, ct] = cbl[ct * 128 : ct * 128 + n]
        cbv[:n, 9 + ct] = 0.5 * cbl[ct * 128 : ct * 128 + n]
    # host params
    hpv = np.zeros((128, 10), np.float32)
    hs = slice(hh * 16, hh * 16 + 16)
    hpv[32:48, 0] = np.asarray(inputs[f"{pre}_dt_bias"], np.float32)[hs]
    hpv[32:48, 1] = -np.exp(np.asarray(inputs[f"{pre}_A_log"], np.float32)[hs])
    Dp = np.asarray(inputs[f"{pre}_Dp"], np.float32)[hs]
    for pr in range(8):
        rows = (np.arange(128) + pr * 128) // 64  # local head of channel
        hpv[:, 2 + pr] = 4.0 * Dp[rows]  # match the 8x gated-y scaling
    return {
        "xt": xtv.astype(bf16),
        "wt": wtv.astype(bf16),
        "mft": mftv.astype(bf16),
        "cw": cwv,
        "cb": cbv,
        "hp": hpv,
    }


def _combine(inputs, results):
    x = np.asarray(inputs["x"], np.float32)
    scale = np.asarray(inputs["layer_scale"], np.float32)
    out = x.copy()
    i = 0
    for b in range(2):
        for d in range(2):
            U0 = np.asarray(results[i]["u"], np.float32)
            s0 = np.asarray(results[i]["s"][0], np.float32)
            U1 = np.asarray(results[i + 1]["u"], np.float32)
            s1 = np.asarray(results[i + 1]["s"][0], np.float32)
            i += 2
            r = 1.0 / np.sqrt((s0 + s1) * 4.0 / 2048.0 + EPS)
            contrib = r[:, None] * (U0 + U1)
            if d == 1:
                contrib = contrib[::-1]
            out[b] += contrib * scale[None, :]
    return out


def _run(inputs, trace=False, core_ids=None):
    from concourse.bass_utils import run_bass_kernel_spmd

    nc = _get_nc()
    in_maps = []
    for b in range(2):
        for d in range(2):
            for hh in range(2):
                in_maps.append(_prep_core_inputs(inputs, b, d, hh))
    if core_ids is None:
        core_ids = list(range(8))
    res = run_bass_kernel_spmd(
        nc, in_maps[: len(core_ids)], core_ids=core_ids, trace=trace
    )
    return res


def kernel(**inputs):
    res = _run(inputs)
    return _combine(inputs, res.results)
